# revision 31
# baseline (speedup 1.0000x reference)
"""Trainium2 Bass kernel for a dense transformer block (B=2, S=2048, D=1024,
H=16, d_ff=4096), sharded over 8 NeuronCores.

Sharding: DP(2 groups over batch) x TP(4 cores over heads) for
LN1/QKV/attention/proj, pipelined per 512-token chunk with a per-chunk bf16
ReduceScatter of the proj partials; then token-parallel MLP (each core:
512 tokens, full MLP weights). Host assembles the 8 per-core outputs.

v2: software-pipelined emission (QKV of chunk ch+1 / LN2 of tile ch-2
emitted before attention of chunk ch), all-bf16 transposes, bn_stats +
Newton-rsqrt LN (no scalar act-table thrash), row-packed score matmuls,
bf16 collectives, fc1 split (tiles 0-2 fill attention gaps).
"""

from contextlib import ExitStack

import numpy as np

import concourse.bacc as bacc
import concourse.mybir as mybir
import concourse.tile as tile
from concourse.bass_utils import run_bass_kernel_spmd
from concourse.masks import make_identity

f32 = mybir.dt.float32
bf16 = mybir.dt.bfloat16
AF = mybir.ActivationFunctionType
OP = mybir.AluOpType

B = 2
S_FULL = 2048
D = 1024
H = 16
HD = 64
DFF_FULL = 4096
LN_EPS = 1e-5
N_CORES = 8
GROUP_FULL = 4
HPC = 4
DJ = D // 128
CS = 512


def build_nc(S=S_FULL, DFF=DFF_FULL, GROUP=GROUP_FULL, n_cores=N_CORES):
    at = bf16
    mt = bf16
    NCH = S // CS
    SL = S // GROUP
    SLT = SL // 128
    NF = DFF // 128
    CSG = CS // GROUP             # rows per core per chunk after RS (=128)
    groups = [list(range(g * GROUP, (g + 1) * GROUP))
              for g in range(n_cores // GROUP)]

    nc = bacc.Bacc("TRN2", target_bir_lowering=False, debug=False,
                   num_devices=n_cores)

    def din(name, shape, dt=f32):
        return nc.dram_tensor(name, shape, dt, kind="ExternalInput").ap()

    x_d = din("x_b", [S, D], bf16)
    xo_d = din("x_own", [SL, D], bf16)
    g1_d = din("g1_m", [128, DJ])
    b1_d = din("b1_m", [128, DJ])
    g2_d = din("g2_m", [128, DJ])
    b2_d = din("b2_m", [128, DJ])
    wq_d = din("wq_m", [128, DJ, 256], at)
    wk_d = din("wk_m", [128, DJ, 256], at)
    wv_d = din("wv_m", [128, DJ, 256], at)
    bq_d = din("bq_m", [128, 2])
    bk_d = din("bk_m", [128, 2])
    bv_d = din("bv_m", [64, 4])
    wp_d = din("wproj_m", [128, 2, D], at)
    wfc_d = din("wfc_m", [NF, 128, DJ, 128], mt)
    bfc_d = din("bfc_m", [128, NF])
    wo_d = din("wout_m", [DFF, D], mt)
    bo_d = din("b_out", [D])
    out_d = nc.dram_tensor("out_s", [SL, D], f32, kind="ExternalOutput").ap()

    with tile.TileContext(nc) as tc, ExitStack() as st0:
        su = st0.enter_context(tc.tile_pool(name="setup", bufs=1))
        wsf = st0.enter_context(tc.tile_pool(name="wsf", bufs=4))
        wso = st0.enter_context(tc.tile_pool(name="wso", bufs=4))
        drp = st0.enter_context(tc.tile_pool(name="dram", bufs=1, space="DRAM"))

        cc_ins = [drp.tile([CS, D], bf16, name=f"cc_in{i}")
                  for i in range(NCH)]
        cc_outs = [drp.tile([CSG, D], bf16, name=f"cc_out{i}")
                   for i in range(NCH)]

        # ---- persistent attention state ----
        per = st0.enter_context(tc.tile_pool(name="attn_per", bufs=1))
        Kt = per.tile([128, 2, S], at, name="Kt")
        Vg = per.tile([128, NCH * 4, HPC, 128], at, name="Vg")
        xP = per.tile([128, SLT, 2, CS], f32, name="xP")
        h2T = per.tile([128, DJ, SL], mt, name="h2T")
        m1T = per.tile([128, NF, SL], mt, name="m1T")

        ident = su.tile([128, 128], f32, name="ident")
        make_identity(nc, ident[:])
        negC = su.tile([128, 1], f32, name="negC")
        nc.vector.memset(negC[:], -4.0)
        nc.gpsimd.memset(Vg[:, :, :, 64:128], 1.0)
        # 4 static causal masks: masks[p][k, q] = 1.0 if q >= k + p*128
        masks = su.tile([128, 4, 512], bf16, name="masks")
        nc.gpsimd.memset(masks[:], 1.0)
        for p in range(4):
            nc.gpsimd.affine_select(
                out=masks[:, p, :], in_=masks[:, p, :],
                compare_op=OP.is_ge, fill=0.0, base=-p * 128,
                pattern=[[1, CS]], channel_multiplier=-1)

        g1v = su.tile([128, DJ], f32, name="g1v")
        nc.sync.dma_start(g1v[:], g1_d)
        b1v = su.tile([128, DJ], f32, name="b1v")
        nc.sync.dma_start(b1v[:], b1_d)
        g2v = su.tile([128, DJ], f32, name="g2v")
        nc.sync.dma_start(g2v[:], g2_d)
        b2v = su.tile([128, DJ], f32, name="b2v")
        nc.sync.dma_start(b2v[:], b2_d)
        bq_sb = su.tile([128, 2], f32, name="bq_sb")
        nc.sync.dma_start(bq_sb[:], bq_d)
        bk_sb = su.tile([128, 2], f32, name="bk_sb")
        nc.sync.dma_start(bk_sb[:], bk_d)
        bv_sb = su.tile([64, 4], f32, name="bv_sb")
        nc.sync.dma_start(bv_sb[:], bv_d)
        bfc_sb = su.tile([128, NF], f32, name="bfc_sb")
        nc.sync.dma_start(bfc_sb[:], bfc_d)

        Wq_sb = su.tile([128, DJ, 256], at, name="Wq_sb")
        Wk_sb = su.tile([128, DJ, 256], at, name="Wk_sb")
        Wv_sb = su.tile([128, DJ, 256], at, name="Wv_sb")
        Wp_sb = su.tile([128, 2, D], at, name="Wp_sb")

        bout_bc = su.tile([128, D], f32, name="bout_bc")
        with tc.tile_pool(name="tmpb", bufs=1) as tb:
            brow = tb.tile([1, D], f32, name="brow")
            nc.sync.dma_start(brow[:], bo_d[None, :])
            nc.gpsimd.partition_broadcast(bout_bc[:], brow[:])

        with ExitStack() as st1:
            # SBUF pools
            p1x = st1.enter_context(tc.tile_pool(name="p1x", bufs=4))
            p1s = st1.enter_context(tc.tile_pool(name="p1s", bufs=2))
            p1n = st1.enter_context(tc.tile_pool(name="p1n", bufs=2))
            p1d = st1.enter_context(tc.tile_pool(name="p1d", bufs=8))
            p1ht = st1.enter_context(tc.tile_pool(name="p1ht", bufs=2))
            pqt = st1.enter_context(tc.tile_pool(name="pqt", bufs=2))
            p2e = st1.enter_context(tc.tile_pool(name="p2e", bufs=4))
            p2t = st1.enter_context(tc.tile_pool(name="p2t", bufs=2))
            p2o = st1.enter_context(tc.tile_pool(name="p2o", bufs=3))
            p4z = st1.enter_context(tc.tile_pool(name="p4z", bufs=2))
            p4s = st1.enter_context(tc.tile_pool(name="p4s", bufs=2))
            p4x = st1.enter_context(tc.tile_pool(name="p4x", bufs=2))
            # PSUM pools: pss(4) + psy(2) + pmm(2) = 8 banks
            pssp = st1.enter_context(
                tc.tile_pool(name="pssp", bufs=2, space="PSUM"))
            psyp = st1.enter_context(
                tc.tile_pool(name="psyp", bufs=2, space="PSUM"))
            pmm = st1.enter_context(
                tc.tile_pool(name="pmm", bufs=2, space="PSUM"))

            Qts = [None] * NCH
            yTs = [None] * NCH
            hTs = [None] * NCH

            def emit_ln1_qkv(ch, xts=None):
                # LN1 stats for the 4 token tiles of chunk ch
                stats = p1s.tile([128, 4, 2], f32, name="stats", tag="stats")
                xcs = []
                for tl in range(4):
                    ti = ch * 4 + tl
                    if xts is not None:
                        xt = xts[tl]
                    else:
                        xt = p1x.tile([128, D], bf16, name="xt", tag="xt")
                        nc.sync.dma_start(
                            xt[:], x_d[ti * 128:(ti + 1) * 128, :])
                    bns = p1s.tile([128, 2, 6], f32, name="bns",
                                   tag=f"bns{tl}")
                    nc.vector.bn_stats(bns[:, 0, :], xt[:, 0:512])
                    nc.vector.bn_stats(bns[:, 1, :], xt[:, 512:1024])
                    nc.vector.bn_aggr(stats[:, tl, :], bns[:])
                    xc = p1x.tile([128, D], bf16, name="xc", tag="xc")
                    nc.vector.tensor_scalar(
                        xc[:], xt[:], stats[:, tl, 0:1], None, OP.subtract)
                    xcs.append(xc)
                # Newton rsqrt on [128,4]: y = rsqrt(var + eps)
                ve = p1n.tile([128, 4], f32, name="ve", tag="ve")
                nc.vector.tensor_scalar(
                    ve[:], stats[:, :, 1:2], LN_EPS, None, OP.add)
                rv = p1n.tile([128, 4], f32, name="rv", tag="rv")
                nc.vector.reciprocal(rv[:], ve[:])
                ys = p1n.tile([128, 4], f32, name="ys", tag="ys")
                nc.vector.tensor_scalar_min(ys[:], rv[:], 1.0)
                tn = p1n.tile([128, 4], f32, name="tn", tag="tn")
                for _ in range(3):
                    nc.vector.tensor_tensor(tn[:], ys[:], ys[:], OP.mult)
                    nc.vector.tensor_tensor(tn[:], tn[:], ve[:], OP.mult)
                    nc.vector.tensor_scalar(
                        tn[:], tn[:], -0.5, 1.5, OP.mult, OP.add)
                    nc.vector.tensor_tensor(ys[:], ys[:], tn[:], OP.mult)
                diags = []
                for tl in range(4):
                    dg = p1d.tile([128, 128], bf16, name="dg", tag="dg")
                    nc.vector.tensor_scalar_mul(
                        dg[:], ident[:], ys[:, tl:tl + 1])
                    diags.append(dg)

                # hT via diag matmuls (bf16), 2 j's per [128,1024] psum
                hT = p1ht.tile([128, DJ, CS], at, name="hT", tag="hT")
                hTs[ch] = hT
                for jh in range(DJ // 2):
                    ptt = pssp.tile([128, 1024], f32, name="pss", tag="pss")
                    for j2 in range(2):
                        j = jh * 2 + j2
                        for tl in range(4):
                            nc.tensor.matmul(
                                ptt[:, j2 * 512 + tl * 128:
                                    j2 * 512 + (tl + 1) * 128],
                                xcs[tl][:, j * 128:(j + 1) * 128],
                                diags[tl][:], start=True, stop=True)
                    for j2 in range(2):
                        j = jh * 2 + j2
                        nc.vector.tensor_scalar(
                            hT[:, j, :], ptt[:, j2 * 512:(j2 + 1) * 512],
                            g1v[:, j:j + 1], b1v[:, j:j + 1],
                            OP.mult, OP.add)

                # QKV
                Qt = pqt.tile([128, 2, CS], at, name="Qt", tag="Qt")
                Qts[ch] = Qt
                for hp in range(2):
                    psq = pmm.tile([128, 512], f32, name="psq", tag="mm")
                    for j in range(DJ):
                        nc.tensor.matmul(
                            psq[:], Wq_sb[:, j, hp * 128:(hp + 1) * 128],
                            hT[:, j, :], start=(j == 0), stop=(j == DJ - 1))
                    nc.vector.tensor_scalar(
                        Qt[:, hp, :], psq[:], bq_sb[:, hp:hp + 1],
                        None, OP.add)
                    psk = pmm.tile([128, 512], f32, name="psk", tag="mm")
                    for j in range(DJ):
                        nc.tensor.matmul(
                            psk[:], Wk_sb[:, j, hp * 128:(hp + 1) * 128],
                            hT[:, j, :], start=(j == 0), stop=(j == DJ - 1))
                    nc.vector.tensor_scalar(
                        Kt[:, hp, ch * CS:(ch + 1) * CS], psk[:],
                        bk_sb[:, hp:hp + 1], None, OP.add)
                for tl in range(4):
                    ti = ch * 4 + tl
                    psv = pmm.tile([128, 512], f32, name="psv", tag="mm")
                    for j in range(DJ):
                        nc.tensor.matmul(
                            psv[:, 0:256],
                            hT[:, j, tl * 128:(tl + 1) * 128],
                            Wv_sb[:, j, :], start=(j == 0),
                            stop=(j == DJ - 1))
                    for h in range(HPC):
                        nc.vector.tensor_copy(
                            Vg[:, ti, h, 0:64], psv[:, h * 64:(h + 1) * 64])

            def emit_ln2_tile(t):
                # z waits on the ReduceScatter; issue on the Activation DGE
                # ring so it cannot head-of-line-block the SP ring's loads.
                z = p4z.tile([128, D], bf16, name="z", tag="z")
                nc.scalar.dma_start(z[:], cc_outs[t][:])
                xre = p4z.tile([128, D], bf16, name="xre", tag="xre")
                nc.scalar.dma_start(xre[:], xo_d[t * 128:(t + 1) * 128, :])
                nc.vector.tensor_tensor(
                    xP[:, t, 0, :], z[:, 0:512], xre[:, 0:512], OP.add)
                nc.vector.tensor_tensor(
                    xP[:, t, 1, :], z[:, 512:1024], xre[:, 512:1024], OP.add)
                bns2 = p4s.tile([128, 2, 6], f32, name="bns2", tag="bns2")
                nc.vector.bn_stats(bns2[:, 0, :], xP[:, t, 0, :])
                nc.vector.bn_stats(bns2[:, 1, :], xP[:, t, 1, :])
                st2t = p4s.tile([128, 2], f32, name="st2t", tag="st2t")
                nc.vector.bn_aggr(st2t[:], bns2[:])
                ve2 = p4s.tile([128, 1], f32, name="ve2", tag="ve2")
                nc.vector.tensor_scalar(
                    ve2[:], st2t[:, 1:2], LN_EPS, None, OP.add)
                rv2 = p4s.tile([128, 1], f32, name="rv2", tag="rv2")
                nc.vector.reciprocal(rv2[:], ve2[:])
                ys2 = p4s.tile([128, 1], f32, name="ys2", tag="ys2")
                nc.vector.tensor_scalar_min(ys2[:], rv2[:], 1.0)
                tn2 = p4s.tile([128, 1], f32, name="tn2", tag="tn2")
                for _ in range(4):
                    nc.vector.tensor_tensor(tn2[:], ys2[:], ys2[:], OP.mult)
                    nc.vector.tensor_tensor(tn2[:], tn2[:], ve2[:], OP.mult)
                    nc.vector.tensor_scalar(
                        tn2[:], tn2[:], -0.5, 1.5, OP.mult, OP.add)
                    nc.vector.tensor_tensor(ys2[:], ys2[:], tn2[:], OP.mult)
                xc2 = p4x.tile([128, D], bf16, name="xc2", tag="xc2")
                nc.vector.tensor_scalar(
                    xc2[:, 0:512], xP[:, t, 0, :], st2t[:, 0:1],
                    None, OP.subtract)
                nc.vector.tensor_scalar(
                    xc2[:, 512:1024], xP[:, t, 1, :], st2t[:, 0:1],
                    None, OP.subtract)
                dg2 = p4x.tile([128, 128], bf16, name="dg2", tag="dg2")
                nc.vector.tensor_scalar_mul(dg2[:], ident[:], ys2[:])
                for jh in range(2):
                    pt2 = pmm.tile([128, 512], f32, name="pt2", tag="mm")
                    for j4 in range(4):
                        j = jh * 4 + j4
                        nc.tensor.matmul(
                            pt2[:, j4 * 128:(j4 + 1) * 128],
                            xc2[:, j * 128:(j + 1) * 128],
                            dg2[:], start=True, stop=True)
                    for j4 in range(4):
                        j = jh * 4 + j4
                        nc.vector.tensor_scalar(
                            h2T[:, j, t * 128:(t + 1) * 128],
                            pt2[:, j4 * 128:(j4 + 1) * 128],
                            g2v[:, j:j + 1], b2v[:, j:j + 1],
                            OP.mult, OP.add)

            def emit_attention(qc):
                q0 = qc * CS
                nkj = (q0 + CS) // 128
                Qt = Qts[qc]
                yT = pqt.tile([128, 2, CS], at, name="yT", tag="yT")
                yTs[qc] = yT
                for hp in range(2):
                    psys = []
                    for h2 in range(2):
                        psy = psyp.tile([128, CS], f32, name="psy", tag="psy")
                        psys.append(psy)
                    first = True
                    for g0 in range(0, nkj, 2):
                        pssab = []
                        for h2 in range(2):
                            pss = pssp.tile([128, 1024], f32, name="pss",
                                            tag="pss")
                            pssab.append(pss)
                        for kk in range(2):
                            kjt = g0 + kk
                            for h2 in range(2):
                                nc.tensor.matmul(
                                    pssab[h2][:, kk * 512:(kk + 1) * 512],
                                    Kt[h2 * 64:(h2 + 1) * 64, hp,
                                       kjt * 128:(kjt + 1) * 128],
                                    Qt[h2 * 64:(h2 + 1) * 64, hp, :],
                                    start=True, stop=True)
                        esab = []
                        for h2 in range(2):
                            es = p2e.tile([128, 1024], at, name="es",
                                          tag="es")
                            nc.scalar.activation(
                                es[:], pssab[h2][:], AF.Exp, bias=negC[:],
                                scale=0.125)
                            esab.append(es)
                        for kk in range(2):
                            kjt = g0 + kk
                            k0 = kjt * 128
                            if k0 >= q0:
                                p = (k0 - q0) // 128
                                for h2 in range(2):
                                    nc.vector.tensor_tensor(
                                        esab[h2][:, kk * 512:(kk + 1) * 512],
                                        esab[h2][:, kk * 512:(kk + 1) * 512],
                                        masks[:, p, :], OP.mult)
                        for kk in range(2):
                            kjt = g0 + kk
                            for h2 in range(2):
                                h = hp * 2 + h2
                                nc.tensor.matmul(
                                    psys[h2][:, :], Vg[:, kjt, h, :],
                                    esab[h2][:, kk * 512:(kk + 1) * 512],
                                    start=first, stop=(kjt == nkj - 1))
                            first = False
                    for h2 in range(2):
                        h = hp * 2 + h2
                        psy = psys[h2]
                        # rows 64:128 of psy all hold the softmax denominator;
                        # copy to SBUF, then DMA shifts it to lanes 0-63
                        # (neither DVE nor DMA can read-shift from PSUM).
                        dsb = p2t.tile([128, CS], f32, name="dsb", tag="dsb")
                        nc.vector.tensor_copy(dsb[64:128, :], psy[64:128, :])
                        dbc = p2t.tile([64, CS], f32, name="dbc", tag="dbc")
                        nc.sync.dma_start(dbc[:], dsb[64:128, :])
                        inv = p2t.tile([64, CS], f32, name="inv", tag="inv")
                        nc.vector.reciprocal_approx_fast(inv[:], dbc[:])
                        if h2 == 0:
                            nc.vector.tensor_tensor(
                                yT[0:64, hp, :], psy[0:64, :], inv[:],
                                OP.mult)
                            nc.vector.tensor_scalar(
                                yT[0:64, hp, :], yT[0:64, hp, :],
                                bv_sb[:, h:h + 1], None, OP.add)
                        else:
                            stg = p2t.tile([64, CS], at, name="stg",
                                           tag="stg")
                            nc.vector.tensor_tensor(
                                stg[:], psy[0:64, :], inv[:], OP.mult)
                            nc.vector.tensor_scalar(
                                stg[:], stg[:], bv_sb[:, h:h + 1],
                                None, OP.add)
                            nc.sync.dma_start(yT[64:128, hp, :], stg[:])

            def emit_proj_rs(qc):
                yT = yTs[qc]
                for tl in range(4):
                    for n in range(2):
                        psp = psyp.tile([128, CS], f32, name="psy",
                                        tag="psy")
                        for hp in range(2):
                            nc.tensor.matmul(
                                psp[:],
                                yT[:, hp, tl * 128:(tl + 1) * 128],
                                Wp_sb[:, hp, n * 512:(n + 1) * 512],
                                start=(hp == 0), stop=(hp == 1))
                        po = p2o.tile([128, 512], bf16, name="po", tag="po")
                        nc.vector.tensor_copy(po[:], psp[:])
                        nc.sync.dma_start(
                            cc_ins[qc][tl * 128:(tl + 1) * 128,
                                       n * 512:(n + 1) * 512], po[:])
                nc.gpsimd.collective_compute(
                    "ReduceScatter", OP.add, replica_groups=groups,
                    ins=[cc_ins[qc][:].opt()],
                    outs=[cc_outs[qc][:].opt()])

            # big weight loads (after chunk-0 x tiles get queue priority)
            def emit_weight_loads():
                nc.sync.dma_start(Wq_sb[:], wq_d)
                nc.sync.dma_start(Wk_sb[:], wk_d)
                nc.sync.dma_start(Wv_sb[:], wv_d)
                nc.sync.dma_start(Wp_sb[:], wp_d)

            def emit_warm(n):
                # dependency-free matmuls: absorb PE idle (RS waits, exp
                # stalls) so HAM stays at full clock; output is never read.
                for _ in range(n // 8):
                    jt = pmm.tile([128, 512], f32, name="jnk", tag="mm")
                    for i in range(8):
                        j2 = 2 * (i % 4)
                        nc.tensor.matmul(
                            jt[:], Wq_sb[:, 0, 0:128],
                            Wq_sb[:, j2:j2 + 2, :], start=True, stop=True)

            # ---------------- pipelined emission ----------------
            # chunk-0 x tiles get queue priority, then the weight loads
            # (weights MUST be written before any reader is emitted)
            xts0 = []
            for tl in range(4):
                xt = p1x.tile([128, D], bf16, name="xt", tag="xt")
                nc.sync.dma_start(xt[:], x_d[tl * 128:(tl + 1) * 128, :])
                xts0.append(xt)
            emit_weight_loads()
            for ch in range(NCH + 1):
                if ch < NCH:
                    emit_ln1_qkv(ch, xts0 if ch == 0 else None)
                if ch >= 2:
                    emit_ln2_tile(ch - 2)
                if ch >= 1:
                    emit_attention(ch - 1)
                    emit_proj_rs(ch - 1)
                    emit_warm(24)

            # fc1 part 1: token tiles 0-2 (fills attention-3 gaps)
            for f in range(NF):
                wf = wsf.tile([128, DJ, 128], mt, name="wf", tag="wf")
                nc.sync.dma_start(wf[:], wfc_d[f])
                psf = pmm.tile([128, 512], f32, name="psf", tag="mm")
                for j in range(DJ):
                    nc.tensor.matmul(
                        psf[:, 0:384], wf[:, j, :], h2T[:, j, 0:384],
                        start=(j == 0), stop=(j == DJ - 1))
                nc.vector.tensor_scalar(
                    m1T[:, f, 0:384], psf[:, 0:384], bfc_sb[:, f:f + 1],
                    0.0, OP.add, OP.max)

            # LN2 for the last tile (needs last RS), then fc1 part 2
            # (wf tiles rotate through the pool, so part 2 re-streams them)
            emit_warm(40)
            emit_ln2_tile(NCH - 1)
            for f in range(NF):
                wf2 = wsf.tile([128, DJ, 128], mt, name="wf2", tag="wf")
                nc.sync.dma_start(wf2[:], wfc_d[f])
                psf = pmm.tile([128, 512], f32, name="psf", tag="mm")
                for j in range(DJ):
                    nc.tensor.matmul(
                        psf[:, 0:128], wf2[:, j, :], h2T[:, j, 384:512],
                        start=(j == 0), stop=(j == DJ - 1))
                nc.vector.tensor_scalar(
                    m1T[:, f, 384:512], psf[:, 0:128], bfc_sb[:, f:f + 1],
                    0.0, OP.add, OP.max)

        # ------------- fc2 -------------
        with tc.tile_pool(name="p6ps", bufs=1, space="PSUM") as p6ps, \
                tc.tile_pool(name="p4o", bufs=2) as p4o:
            pso = [[p6ps.tile([128, 512], f32, name=f"pso_{tl}_{n}")
                    for n in range(2)] for tl in range(SLT)]
            for f in range(NF):
                wo = wso.tile([128, D], mt, name="wo", tag="wo")
                nc.sync.dma_start(
                    wo[:], wo_d[f * 128:(f + 1) * 128, :])
                for tl in range(SLT):
                    for n in range(2):
                        nc.tensor.matmul(
                            pso[tl][n][:],
                            m1T[:, f, tl * 128:(tl + 1) * 128],
                            wo[:, n * 512:(n + 1) * 512],
                            start=(f == 0), stop=(f == NF - 1))
            for tl in range(SLT):
                for n in range(2):
                    ot = p4o.tile([128, 512], f32, name="ot", tag="ot")
                    nc.vector.tensor_tensor(
                        ot[:], pso[tl][n][:],
                        xP[:, tl, n, :], OP.add)
                    nc.vector.tensor_tensor(
                        ot[:], ot[:],
                        bout_bc[:, n * 512:(n + 1) * 512], OP.add)
                    nc.sync.dma_start(
                        out_d[tl * 128:(tl + 1) * 128,
                              n * 512:(n + 1) * 512], ot[:])
    nc.compile()
    return nc


def own_token_idx(t, S=S_FULL, GROUP=GROUP_FULL):
    CSG = CS // GROUP
    return np.concatenate([
        np.arange(qc * CS + t * CSG, qc * CS + (t + 1) * CSG)
        for qc in range(S // CS)])


def marshal_inputs(x, ln1_g, ln1_b, ln2_g, ln2_b, W_qkv, b_qkv, W_proj,
                   b_proj, W_fc, b_fc, W_out, b_out,
                   S=S_FULL, DFF=DFF_FULL, GROUP=GROUP_FULL,
                   n_cores=N_CORES):
    NF = DFF // 128
    import ml_dtypes
    adt = ml_dtypes.bfloat16
    mdt = ml_dtypes.bfloat16

    def f32c(a):
        return np.ascontiguousarray(a, dtype=np.float32)

    def ac(a):
        return np.ascontiguousarray(a, dtype=adt)

    def mc(a):
        return np.ascontiguousarray(a, dtype=mdt)

    def ln_m(v):
        return f32c(v.reshape(DJ, 128).T)

    base = {
        "g1_m": ln_m(ln1_g), "b1_m": ln_m(ln1_b),
        "g2_m": ln_m(ln2_g), "b2_m": ln_m(ln2_b),
        "bfc_m": f32c(b_fc.reshape(NF, 128).T),
        "wfc_m": mc(W_fc.reshape(DJ, 128, NF, 128).transpose(2, 1, 0, 3)),
        "wout_m": mc(W_out),
        "b_out": f32c(b_out),
    }
    in_maps = []
    for c in range(n_cores):
        g, t = c // GROUP, c % GROUP
        cs, ce = t * 256, (t + 1) * 256
        wq = W_qkv[:, cs:ce]
        wk = W_qkv[:, D + cs:D + ce]
        wv = W_qkv[:, 2 * D + cs:2 * D + ce]
        bq = b_qkv[cs:ce]
        bk = b_qkv[D + cs:D + ce]
        bv = b_qkv[2 * D + cs:2 * D + ce]
        wp = W_proj[cs:ce, :]
        m = dict(base)
        m["x_b"] = ac(x[g])
        m["x_own"] = ac(x[g][own_token_idx(t, S, GROUP)] + b_proj)
        m["wq_m"] = ac(wq.reshape(DJ, 128, 256).transpose(1, 0, 2))
        m["wk_m"] = ac(wk.reshape(DJ, 128, 256).transpose(1, 0, 2))
        m["wv_m"] = ac(wv.reshape(DJ, 128, 256).transpose(1, 0, 2))
        m["bq_m"] = f32c(bq.reshape(2, 128).T)
        m["bk_m"] = f32c(bk.reshape(2, 128).T)
        m["bv_m"] = f32c(bv.reshape(4, 64).T)
        m["wproj_m"] = ac(
            wp.reshape(2, 2, 64, D).transpose(1, 2, 0, 3).reshape(128, 2, D))
        in_maps.append(m)
    return in_maps


_NC_CACHE = {}


def _get_nc():
    if "nc" not in _NC_CACHE:
        _NC_CACHE["nc"] = build_nc()
    return _NC_CACHE["nc"]


def kernel(**inputs):
    inputs = {k: np.asarray(v, dtype=np.float32) for k, v in inputs.items()}
    nc = _get_nc()
    in_maps = marshal_inputs(**inputs)
    r = run_bass_kernel_spmd(nc, in_maps, core_ids=list(range(N_CORES)))
    out = np.empty((B, S_FULL, D), np.float32)
    for c in range(N_CORES):
        g, t = c // GROUP_FULL, c % GROUP_FULL
        out[g, own_token_idx(t), :] = r.results[c]["out_s"]
    return out
